# revision 4
# baseline (speedup 1.0000x reference)
"""DeepLSTMDecoderLayer Trainium2 kernel: data-parallel over batch on 8 NeuronCores.

Per core (BC=8 batches): projections -> dual attention (cross+self) ->
out-projection -> gate precompute -> LSTM scan -> FFN.  No collectives.
All matmuls bf16 (fp32 PSUM accum); softmax, LN stats and LSTM cell state fp32.

Layouts (per core):
  cols "(b,t)": col = b*T + t   -- projections / attention / precompute rows
  cols "(t,b)": col = t*BC + b  -- hidT (transposed hiddens) / FFN rows
"""
import sys
sys.path.insert(0, '/opt/trn_rl_repo')
import numpy as np
import ml_dtypes

T, B, H, S, HEADS, FILTER = 128, 64, 1024, 128, 16, 4096
HD = H // HEADS
N_CORES = 8
BC = B // N_CORES
R = BC * T                  # rows per core (1024)
KH = H // 128               # 8 tiles over H
KF = FILTER // 128          # 32 tiles over FILTER
EPS = 1e-6
SM_SCALE = float(HD) ** -0.5

_CACHE = {}


def _build():
    import concourse.bacc as bacc
    import concourse.mybir as mybir
    import concourse.tile as tile
    from concourse.masks import make_identity

    f32 = mybir.dt.float32
    bf16 = mybir.dt.bfloat16
    AF = mybir.ActivationFunctionType
    MULT = mybir.AluOpType.mult
    ADD = mybir.AluOpType.add

    nc = bacc.Bacc(None, target_bir_lowering=False, debug=False, num_devices=N_CORES)

    xT_e = nc.declare_dram_parameter("xT", [H, R], bf16, isOutput=False)
    memT_e = nc.declare_dram_parameter("memT", [H, R], bf16, isOutput=False)
    wq_e = nc.declare_dram_parameter("wq", [H, H], bf16, isOutput=False)
    wk_e = nc.declare_dram_parameter("wk", [H, H], bf16, isOutput=False)
    wv_e = nc.declare_dram_parameter("wv", [H, H], bf16, isOutput=False)
    wo_e = nc.declare_dram_parameter("wo", [H, H], bf16, isOutput=False)
    wxc_e = nc.declare_dram_parameter("wxc", [2 * H, 4 * H], bf16, isOutput=False)
    wh_e = nc.declare_dram_parameter("wh", [H, 4 * H], bf16, isOutput=False)
    w1_e = nc.declare_dram_parameter("w1", [H, FILTER], bf16, isOutput=False)
    w2_e = nc.declare_dram_parameter("w2", [FILTER, H], bf16, isOutput=False)
    xc_dram = nc.dram_tensor("xc_scratch", [R, 4 * H], bf16)     # rows (b,t)
    out_e = nc.declare_dram_parameter("out", [R, H], f32, isOutput=True)   # rows (t,b)
    cf_e = nc.declare_dram_parameter("cf", [BC, H], f32, isOutput=True)
    hf_e = nc.declare_dram_parameter("hf", [BC, H], f32, isOutput=True)

    with tile.TileContext(nc) as tc:
        with tc.tile_pool(name="persist", bufs=1) as pers:
            id128 = pers.tile([128, 128], bf16)
            make_identity(nc, id128[:])
            idK = pers.tile([128, BC], bf16)
            zf = pers.tile([128, BC], f32)
            nc.any.memset(zf[:], 0.0)
            nc.vector.tensor_copy(idK[:], zf[:])
            make_identity(nc, idK[0:BC, :], nomemset=True)
            hidT = pers.tile([128, KH, R], bf16)     # transposed hiddens, cols (t,b)

            # ================= phases 0-3 =================
            with tc.tile_pool(name="attn_big", bufs=1) as big:
                qT = big.tile([128, KH, R], bf16)
                kxT = big.tile([128, KH, R], bf16)
                kmT = big.tile([128, KH, R], bf16)
                vx = big.tile([128, KH, R], bf16)
                vm = big.tile([128, KH, R], bf16)
                oT = big.tile([128, KH, R], bf16)
                cT = big.tile([128, KH, R], bf16)
                xTs = big.tile([128, KH, R], bf16)
                nc.sync.dma_start(xTs[:], xT_e.ap().rearrange("(k p) r -> p k r", p=128))

                # out_sb[:,m,:] = w[:,128m:...]^T @ rhs  -- w streamed once (m-outer, k cached)
                def projection(w_ext, rhs_sb, out_sb):
                    with tc.tile_pool(name="pr_w", bufs=KH + 1) as wp, \
                         tc.tile_pool(name="pr_ps", bufs=4, space="PSUM") as pp:
                        for m in range(KH):
                            wts = []
                            for k in range(KH):
                                wt = wp.tile([128, 128], bf16, tag="w")
                                nc.sync.dma_start(wt[:], w_ext.ap()[128 * k:128 * (k + 1), 128 * m:128 * (m + 1)])
                                wts.append(wt)
                            for n in range(2):
                                ps = pp.tile([128, 512], f32)
                                for k in range(KH):
                                    nc.tensor.matmul(ps[:], wts[k][:], rhs_sb[:, k, 512 * n:512 * (n + 1)],
                                                     start=(k == 0), stop=(k == KH - 1))
                                nc.scalar.copy(out_sb[:, m, 512 * n:512 * (n + 1)], ps[:])

                # v = rhs^T @ wv: n-outer, k-tiles of wv cached
                def vproj(rhs_sb, out_sb):
                    with tc.tile_pool(name="vp_w", bufs=KH + 1) as wp, \
                         tc.tile_pool(name="vp_ps", bufs=4, space="PSUM") as pp:
                        for n in range(2):
                            wts = []
                            for k in range(KH):
                                wt = wp.tile([128, 512], bf16, tag="wv")
                                nc.sync.dma_start(wt[:], wv_e.ap()[128 * k:128 * (k + 1), 512 * n:512 * (n + 1)])
                                wts.append(wt)
                            for m in range(KH):
                                ps = pp.tile([128, 512], f32)
                                for k in range(KH):
                                    nc.tensor.matmul(ps[:], rhs_sb[:, k, 128 * m:128 * (m + 1)], wts[k][:],
                                                     start=(k == 0), stop=(k == KH - 1))
                                nc.scalar.copy(out_sb[:, m, 512 * n:512 * (n + 1)], ps[:])

                with tc.tile_pool(name="mem_in", bufs=1) as memp:
                    memTs = memp.tile([128, KH, R], bf16)
                    nc.sync.dma_start(memTs[:], memT_e.ap().rearrange("(k p) r -> p k r", p=128))
                    projection(wk_e, memTs, kmT)
                    vproj(memTs, vm)
                projection(wq_e, xTs, qT)
                projection(wk_e, xTs, kxT)
                vproj(xTs, vx)

                # -------- attention core --------
                with tc.tile_pool(name="at_sb", bufs=4) as asb, \
                     tc.tile_pool(name="at_lg", bufs=2, space="PSUM") as plg, \
                     tc.tile_pool(name="at_et", bufs=2, space="PSUM") as pet, \
                     tc.tile_pool(name="at_av", bufs=2, space="PSUM") as pav:
                    for b in range(BC):
                        for h in range(HEADS):
                            pb = 64 * (h % 2)
                            m = h // 2
                            q_sl = qT[pb:pb + 64, m, 128 * b:128 * (b + 1)]
                            av = pav.tile([128, 128], f32, tag="av")
                            for att, (kT_sb, v_sb) in enumerate(((kmT, vm), (kxT, vx))):
                                k_sl = kT_sb[pb:pb + 64, m, 128 * b:128 * (b + 1)]
                                lg = plg.tile([128, 128], f32, tag="lg")
                                nc.tensor.matmul(lg[:], q_sl, k_sl, start=True, stop=True,
                                                 tile_position=(pb, 0))
                                ex = asb.tile([128, 128], f32, tag="ex")
                                dsum = asb.tile([128, 1], f32, tag="d")
                                nc.scalar.activation(ex[:], lg[:], AF.Exp, scale=SM_SCALE,
                                                     accum_out=dsum[:])
                                rcp = asb.tile([128, 1], f32, tag="r")
                                nc.vector.reciprocal(rcp[:], dsum[:])
                                en = asb.tile([128, 128], bf16, tag="en")
                                nc.vector.tensor_scalar_mul(en[:], ex[:], rcp[:])
                                etp = pet.tile([128, 128], bf16, tag="et")
                                nc.tensor.transpose(etp[:], en[:], id128[:])
                                ets = asb.tile([128, 128], bf16, tag="ets")
                                nc.vector.tensor_copy(ets[:], etp[:])
                                nc.tensor.matmul(av[pb:pb + 64, :], v_sb[:, b, 64 * h:64 * (h + 1)],
                                                 ets[:], start=(att == 0), stop=(att == 1),
                                                 tile_position=(0, pb))
                            nc.scalar.copy(oT[pb:pb + 64, m, 128 * b:128 * (b + 1)], av[pb:pb + 64, :])

                # -------- out-projection --------
                projection(wo_e, oT, cT)

                # -------- gate precompute: xc_pre = [x;c] @ wxc, rows (b,t) --------
                with tc.tile_pool(name="pc_w", bufs=2 * KH + 1) as pw, \
                     tc.tile_pool(name="pc_ps", bufs=4, space="PSUM") as pps, \
                     tc.tile_pool(name="pc_ev", bufs=4) as pev:
                    for n in range(8):
                        wts = []
                        for k in range(2 * KH):
                            wt = pw.tile([128, 512], bf16, tag="wxc")
                            nc.sync.dma_start(wt[:], wxc_e.ap()[128 * k:128 * (k + 1), 512 * n:512 * (n + 1)])
                            wts.append(wt)
                        for m in range(KH):
                            ps = pps.tile([128, 512], f32)
                            for k in range(2 * KH):
                                lh = (xTs[:, k, 128 * m:128 * (m + 1)] if k < KH
                                      else cT[:, k - KH, 128 * m:128 * (m + 1)])
                                nc.tensor.matmul(ps[:], lh, wts[k][:], start=(k == 0),
                                                 stop=(k == 2 * KH - 1))
                            ev = pev.tile([128, 512], bf16, tag="ev")
                            nc.scalar.copy(ev[:], ps[:])
                            nc.sync.dma_start(xc_dram.ap()[128 * m:128 * (m + 1), 512 * n:512 * (n + 1)], ev[:])

            # ================= phase 4: LSTM scan =================
            xc_v = xc_dram.ap().rearrange("(b t) n -> b t n", t=T)
            with tc.tile_pool(name="sc_w", bufs=1) as swp, \
                 tc.tile_pool(name="sc_st", bufs=1) as sst, \
                 tc.tile_pool(name="sc_g", bufs=2, space="PSUM") as sgp, \
                 tc.tile_pool(name="sc_t", bufs=2, space="PSUM") as stp:
                whs = swp.tile([128, KH, 4 * H], bf16)
                nc.sync.dma_start(whs[:], wh_e.ap().rearrange("(k p) n -> p k n", p=128))
                xcb = [swp.tile([128, 4 * H], bf16, tag=f"xcb{i}", name=f"xcb{i}") for i in range(2)]
                zrow = swp.tile([128, 512], f32, tag="zrow")
                nc.any.memset(zrow[:], 0.0)
                for i in range(2):
                    for n in range(8):
                        nc.vector.tensor_copy(xcb[i][:, 512 * n:512 * (n + 1)], zrow[:])
                c_sb = sst.tile([BC, H], f32, tag="c")
                nc.any.memset(c_sb[:], 0.0)
                gate = [sst.tile([BC, H], f32, tag=f"g{g}", name=f"gate{g}") for g in range(4)]  # i j f o
                tanh_c = sst.tile([BC, H], f32, tag="tc")
                t1 = sst.tile([BC, H], f32, tag="t1")
                t2 = sst.tile([BC, H], f32, tag="t2")
                h_bf = sst.tile([BC, H], bf16, tag="hbf")
                bnt = sst.tile([BC, 4, 12], f32, tag="bnt")
                mv = sst.tile([BC, 4, 2], f32, tag="mv")
                inv = sst.tile([BC, 4], f32, tag="inv")
                nmi = sst.tile([BC, 4], f32, tag="nmi")
                sd = sst.tile([BC, 4], f32, tag="sd")
                epst = sst.tile([BC, 1], f32, tag="epst")
                nc.any.memset(epst[:], EPS)

                for t in range(T):
                    xct = xcb[t % 2]
                    nc.sync.dma_start(xct[0:BC, :], xc_v[:, t, :])
                    gp = sgp.tile([128, H], f32, tag="gates")
                    for cc in range(2):
                        for k in range(-1, KH if t > 0 else 0):
                            for g in range(4):
                                n0 = 1024 * g + 512 * cc
                                dst = gp[32 * g:32 * g + BC, 512 * cc:512 * (cc + 1)]
                                if k < 0:
                                    nc.tensor.matmul(dst, idK[:], xct[:, n0:n0 + 512],
                                                     start=True, stop=(t == 0),
                                                     tile_position=(0, 32 * g))
                                else:
                                    nc.tensor.matmul(dst, hidT[:, k, BC * (t - 1):BC * t],
                                                     whs[:, k, n0:n0 + 512],
                                                     start=False, stop=(k == KH - 1),
                                                     tile_position=(0, 32 * g))
                    for g in range(4):
                        src = gp[32 * g:32 * g + BC, :]
                        nc.vector.bn_stats(bnt[:, g, 0:6], src[:, 0:512])
                        nc.vector.bn_stats(bnt[:, g, 6:12], src[:, 512:1024])
                        nc.vector.bn_aggr(mv[:, g, :], bnt[:, g, :])
                    nc.scalar.activation(sd[:], mv[:, :, 1], AF.Sqrt, bias=epst[:])
                    nc.vector.reciprocal(inv[:], sd[:])
                    nc.vector.tensor_tensor(nmi[:], mv[:, :, 0], inv[:], MULT)
                    nc.vector.tensor_scalar_mul(nmi[:], nmi[:], -1.0)
                    for g, fn in ((0, AF.Sigmoid), (1, AF.Tanh), (2, AF.Sigmoid), (3, AF.Sigmoid)):
                        nc.scalar.activation(gate[g][:], gp[32 * g:32 * g + BC, :], fn,
                                             scale=inv[:, g:g + 1], bias=nmi[:, g:g + 1])
                    nc.vector.tensor_tensor(t1[:], gate[0][:], gate[1][:], MULT)
                    nc.vector.tensor_tensor(t2[:], gate[2][:], c_sb[:], MULT)
                    nc.vector.tensor_tensor(c_sb[:], t1[:], t2[:], ADD)
                    nc.scalar.activation(tanh_c[:], c_sb[:], AF.Tanh)
                    nc.vector.tensor_tensor(h_bf[:], gate[3][:], tanh_c[:], MULT)
                    tp = stp.tile([128, 64], bf16, tag="tp")
                    for k in range(KH):
                        nc.tensor.transpose(tp[:, BC * k:BC * (k + 1)],
                                            h_bf[:, 128 * k:128 * (k + 1)], id128[0:BC, 0:BC])
                    nc.vector.tensor_copy(hidT[:, :, BC * t:BC * (t + 1)],
                                          tp[:].rearrange("p (k b) -> p k b", b=BC))
                    if t == T - 1:
                        nc.sync.dma_start(cf_e.ap(), c_sb[:])
                        hf_sb = sst.tile([BC, H], f32, tag="hff")
                        nc.vector.tensor_tensor(hf_sb[:], gate[3][:], tanh_c[:], MULT)
                        nc.sync.dma_start(hf_e.ap(), hf_sb[:])

            # ================= phase 5: FFN =================
            with tc.tile_pool(name="ff_f1", bufs=1) as f1p:
                f1T = f1p.tile([128, KF, R], bf16)
                with tc.tile_pool(name="ff_w1", bufs=KH + 1) as fwp, \
                     tc.tile_pool(name="ff_ps1", bufs=4, space="PSUM") as fps:
                    for m in range(KF):
                        wts = []
                        for k in range(KH):
                            wt = fwp.tile([128, 128], bf16, tag="w1t")
                            nc.sync.dma_start(wt[:], w1_e.ap()[128 * k:128 * (k + 1), 128 * m:128 * (m + 1)])
                            wts.append(wt)
                        for n in range(2):
                            ps = fps.tile([128, 512], f32)
                            for k in range(KH):
                                nc.tensor.matmul(ps[:], wts[k][:], hidT[:, k, 512 * n:512 * (n + 1)],
                                                 start=(k == 0), stop=(k == KH - 1))
                            nc.scalar.activation(f1T[:, m, 512 * n:512 * (n + 1)], ps[:], AF.Relu)
                with tc.tile_pool(name="ff_w2", bufs=KF + 1) as fwp2, \
                     tc.tile_pool(name="ff_ps2", bufs=4, space="PSUM") as fps2, \
                     tc.tile_pool(name="ff_ev", bufs=4) as fev:
                    for n in range(2):
                        wts = []
                        for k in range(KF):
                            wt = fwp2.tile([128, 512], bf16, tag="w2t")
                            nc.sync.dma_start(wt[:], w2_e.ap()[128 * k:128 * (k + 1), 512 * n:512 * (n + 1)])
                            wts.append(wt)
                        for m in range(KH):
                            ps = fps2.tile([128, 512], f32)
                            for k in range(KF):
                                nc.tensor.matmul(ps[:], f1T[:, k, 128 * m:128 * (m + 1)], wts[k][:],
                                                 start=(k == 0), stop=(k == KF - 1))
                            ev = fev.tile([128, 512], f32, tag="ev2")
                            nc.vector.tensor_copy(ev[:], ps[:])
                            nc.sync.dma_start(out_e.ap()[128 * m:128 * (m + 1), 512 * n:512 * (n + 1)], ev[:])

    nc.compile()
    return nc


def _prep_inputs(inputs):
    bf = ml_dtypes.bfloat16
    x = np.asarray(inputs["x"])
    mem = np.asarray(inputs["memory"])
    wg = np.asarray(inputs["w_gates"])
    shared = {
        "wq": np.asarray(inputs["wq"]).astype(bf),
        "wk": np.asarray(inputs["wk"]).astype(bf),
        "wv": np.asarray(inputs["wv"]).astype(bf),
        "wo": np.asarray(inputs["wo"]).astype(bf),
        "wxc": np.ascontiguousarray(wg[:2 * H]).astype(bf),
        "wh": np.ascontiguousarray(wg[2 * H:]).astype(bf),
        "w1": np.asarray(inputs["w1"]).astype(bf),
        "w2": np.asarray(inputs["w2"]).astype(bf),
    }
    per_core = []
    for c in range(N_CORES):
        bs = slice(BC * c, BC * (c + 1))
        xT = np.ascontiguousarray(x[:, bs, :].transpose(2, 1, 0).reshape(H, R)).astype(bf)
        memT = np.ascontiguousarray(mem[:, bs, :].transpose(2, 1, 0).reshape(H, R)).astype(bf)
        per_core.append({"xT": xT, "memT": memT, **shared})
    return per_core


def kernel(**inputs):
    from concourse.bass_utils import run_bass_kernel_spmd
    for nm in ("bq", "bk", "bv", "bo", "b_gates", "b1", "b2", "ln_bias",
               "src_bias", "tgt_bias"):
        assert not np.any(np.asarray(inputs[nm])), f"{nm} must be zero"
    assert np.all(np.asarray(inputs["ln_scale"]) == 1.0)

    if "nc" not in _CACHE:
        _CACHE["nc"] = _build()
    nc = _CACHE["nc"]
    in_maps = _prep_inputs(inputs)
    res = run_bass_kernel_spmd(nc, in_maps, list(range(N_CORES))).results

    out = np.empty((T, B, H), np.float32)
    cf = np.empty((B, H), np.float32)
    hf = np.empty((B, H), np.float32)
    for c in range(N_CORES):
        bs = slice(BC * c, BC * (c + 1))
        out[:, bs, :] = res[c]["out"].reshape(T, BC, H)
        cf[bs] = res[c]["cf"]
        hf[bs] = res[c]["hf"]
    return out, cf, hf
